# revision 1
# baseline (speedup 1.0000x reference)
"""Trainium2 Bass kernel for nn_Net_63754494542044.

Data-parallel over 8 NeuronCores (8 B-samples each). Host pre-packs
conv1 im2col / conv weights / RoIAlign grid tables; device runs
conv1 -> conv2 -> RoIAlign gather+bilinear -> fc0/emb/red -> 8 GNN rollouts.
"""
import sys
sys.path.insert(0, '/opt/trn_rl_repo')
import numpy as np
from contextlib import ExitStack
import concourse.bass as bass
import concourse.tile as tile
from concourse import mybir
from concourse.bass_utils import run_bass_kernel_spmd

# Walrus wait-slot limits: CTRL-encoded (Drain/NoOp) = 1; others appear
# limited too on this build -- split conservatively.
def split_drain_waits(nc, max_waits=1, max_waits_other=1):
    for fn in nc.m.functions:
        for bb in fn.blocks:
            insts = bb.instructions
            i = 0
            while i < len(insts):
                inst = insts[i]
                si = getattr(inst, 'sync_info', None)
                lim = max_waits if isinstance(inst, (mybir.InstDrain, mybir.InstNoOp)) else max_waits_other
                if si is not None and si.on_wait and len(si.on_wait) > lim:
                    waits = list(si.on_wait)
                    keep = waits[-lim:]
                    extra = waits[:-lim]
                    new_nops = []
                    for k in range(0, len(extra), max_waits):
                        chunk = extra[k:k + max_waits]
                        nop = mybir.InstNoOp(
                            name=nc.get_next_instruction_name(),
                            engine=inst.engine,
                        )
                        nop.sync_info = mybir.SyncInfo(on_wait=chunk, on_update=[])
                        nc.register_instruction(nop)
                        new_nops.append(nop)
                    inst.sync_info = mybir.SyncInfo(on_wait=keep, on_update=list(si.on_update))
                    insts[i:i] = new_nops
                    i += len(new_nops)
                i += 1


B, T, N = 64, 4, 6
IMG, CIN = 128, 3
VE, D, P = 64, 256, 4
SCALE = 0.25
NCORE = 8
BC = B // NCORE          # 8 samples per core
NIMG = BC * T            # 32 images per core
NROI = BC * T * N        # 192 rois per core
NROW = BC * N            # 48 gnn rows per core
NPT = NROI * 16          # 3072 sample points per core
NG = 24                  # gather groups


# ---------------- conv1 im2col (host) ----------------
# conv1: 3->64, 3x3, stride2, SAME on 128x128 -> 64x64.
# 2-px-packed output: out pair (oy, j) covers ox = 2j, 2j+1.
# K=45 rows: (rowtap rt in 0..2) x (coltap ct in 0..4) x (ci in 0..2)
#   input row for out oy: rt0: 2*oy-1, rt1: 2*oy, rt2: 2*oy+1
#   input col for out pair j: ct: 4j-1, 4j, 4j+1, 4j+2, 4j+3
def conv1_im2col_host(x):  # x [nimg, 3, 128, 128] fp32
    nimg = x.shape[0]
    xp = np.pad(x, ((0, 0), (0, 0), (0, 1), (0, 1)))  # SAME stride2: pad bottom/right only
    cols = np.empty((45, nimg, 64, 32), np.float32)
    k = 0
    for rt in range(3):
        for ct in range(5):
            for ci in range(3):
                # row = 2*oy + rt ; col = 4*j + ct
                cols[k] = xp[:, ci, rt:rt + 127:2, ct:ct + 125:4]
                k += 1
    return cols  # [45, nimg, 64, 32]


def conv1_weights_host(w_conv1):  # [64, 3, 3, 3]
    # W2 [45, 128]: col m = px*64 + oc ... out(oy, 2j+px) uses taps:
    #   orig tap (dy, dx): input row 2oy+dy-1 -> rt = dy ; input col 2(2j+px)+dx-1 = 4j + (2px+dx-1) -> ct = 2px+dx-1
    W2 = np.zeros((45, 128), np.float32)
    for px in range(2):
        for oc in range(64):
            m = px * 64 + oc
            for dy in range(3):
                for dx in range(3):
                    ct = 2 * px + dx
                    assert 0 <= ct <= 4  # input col = 4j + ct (no left pad)
                    for ci in range(3):
                        W2[(dy * 5 + ct) * 3 + ci, m] = w_conv1[oc, ci, dy, dx]
    return W2


def conv1_host(x, w_conv1, b_conv1):
    """Mirror of device conv1: returns feat1 [nimg, 64, 64, 64] (pre-relu + bias)."""
    cols = conv1_im2col_host(x)          # [45, nimg, 64, 32]
    W2 = conv1_weights_host(w_conv1)     # [45, 128]
    out = np.einsum('kf,kc->cf', cols.reshape(45, -1), W2)  # [128, nimg*64*32]
    out = out.reshape(2, 64, -1, 64, 32)  # [px, oc, img, oy, j]
    feat1 = np.empty((x.shape[0], 64, 64, 64), np.float32)
    feat1[..., 0::2] = np.transpose(out[0], (1, 0, 2, 3))
    feat1[..., 1::2] = np.transpose(out[1], (1, 0, 2, 3))
    feat1 += b_conv1[None, :, None, None]
    return feat1


# ---------------- conv2 weights (host) ----------------
# feat1_ph partitions: (px_in*64 + ci'), free (img, py, Y, X) halo X,Y in -1..31.
# conv2 out pair (oy2, j2): outs o1=2*j2, o2=2*j2+1 ; M col = pxo*64 + oc.
# 9 matmuls: rowtap r in {py0[Y], py1[Y-1], py1[Y]} x colgrp g in {X=j2 pair(K128), X=j2+?...}
# col groups: g0: pair (px0[Xa], px1[Xa]) Xa = j2? ... define by original dx:
#   out ox2: input x = 2*ox2 + dx - 1
#   for o1=2j2: x = 4j2-1, 4j2, 4j2+1 -> (px,X): (1, 2j2-1), (0, 2j2), (1, 2j2)
#   for o2=2j2+1: x = 4j2+1, 4j2+2, 4j2+3 -> (1, 2j2), (0, 2j2+1), (1, 2j2+1)
# X taps: px0: {2j2, 2j2+1} ; px1: {2j2-1, 2j2, 2j2+1}
# col groups (relative X offset from base 2j2):
#   gA: K128 = (px0[2j2], px1[2j2])        -> X offset 0, both phases
#   gB: K128 = (px0[2j2+1], px1[2j2+1])    -> X offset +1, both phases
#   gC: K64  = px1[2j2-1]                  -> X offset -1, px1 only
# rowtaps r (input y = 2*oy2 + dy - 1):
#   dy0: y = 2oy2-1 -> (py1, Y=oy2-1) ; dy1: y=2oy2 -> (py0, Y=oy2) ; dy2: y=2oy2+1 -> (py1, Y=oy2)
def conv2_weights_host(w_conv2):  # [64, 64, 3, 3]
    # Wb[r][g]: gA/gB: [128, 128] (partition = pxi*64+ci), gC: [64, 128]
    # dy maps to rowtap r directly (r=0: dy=0 ; r=1: dy=1 ; r=2: dy=2)
    Wb = [[np.zeros((128, 128), np.float32) for _ in range(2)] + [np.zeros((64, 128), np.float32)]
          for _ in range(3)]
    for pxo in range(2):          # which output in the pair (o = 2j2+pxo)
        for oc in range(64):
            m = pxo * 64 + oc
            for dy in range(3):
                for dx in range(3):
                    x_off = 2 * pxo + dx     # input x = 4j2 + x_off, x_off in 0..4
                    pxi = x_off % 2
                    Xrel = x_off // 2        # in {0, 1, 2}
                    for ci in range(64):
                        if Xrel < 2:
                            Wb[dy][Xrel][pxi * 64 + ci, m] += w_conv2[oc, ci, dy, dx]
                        else:
                            assert pxi == 0
                            Wb[dy][2][ci, m] += w_conv2[oc, ci, dy, dx]
    return Wb


def conv2_host(feat1r, w_conv2, b_conv2):
    """feat1r: relu'd feat1 [nimg, 64, 64, 64]. Returns feat2 [nimg, 64, 32, 32] pre-relu."""
    nimg = feat1r.shape[0]
    # build feat1_ph with halo: [128 part (pxi*64+ci), img, py, Y(-1..31), X(-1..31)]
    ph = np.zeros((128, nimg, 2, 33, 33), np.float32)  # halo at Y=32, X=32
    f = feat1r  # [img, ci, y, x]
    for pxi in range(2):
        for py in range(2):
            ph[pxi * 64:pxi * 64 + 64, :, py, :32, :32] = np.transpose(
                f[:, :, py::2, pxi::2], (1, 0, 2, 3))
    Wb = conv2_weights_host(w_conv2)
    out = np.zeros((128, nimg, 32, 16), np.float32)  # [(pxo,oc), img, oy2, j2]
    # rowtap dy: input y = 2*oy2 + dy -> (py = dy&1, Y = oy2 + dy//2)
    for dy in range(3):
        py, Yoff = dy % 2, dy // 2
        for g in range(3):
            W = Wb[dy][g]
            Ysl = slice(Yoff, Yoff + 32)
            Xidx = g + 2 * np.arange(16)   # X = 2*j2 + Xrel ... stored X index = that
            rhs = ph[:, :, py, Ysl, :][:, :, :, Xidx]  # [128 or .., img, 32, 16]
            if g == 2:
                rhs = rhs[:64]
            out += np.einsum('km,kijx->mijx', W, rhs)
    feat2 = np.empty((nimg, 64, 32, 32), np.float32)
    feat2[..., 0::2] = np.transpose(out[:64], (1, 0, 2, 3))
    feat2[..., 1::2] = np.transpose(out[64:], (1, 0, 2, 3))
    return feat2 + b_conv2[None, :, None, None]


# ---------------- RoIAlign grid (host) ----------------
def roi_grid_host(rois):  # rois [NROI, 5] fp32 (batch-local; bidx = local img idx)
    """Returns idx int32 [NPT, 2] (row-gather indices, row=(img,y,j2) width 128),
    weights w4 [NPT, 4] fp32 (w00,w01,w10,w11 order: (y0x0, y0x1, y1x0, y1x1)),
    parity [NPT] (x0&1)."""
    nroi = rois.shape[0]
    W = H = 32
    x1 = rois[:, 1] * SCALE; y1 = rois[:, 2] * SCALE
    x2 = rois[:, 3] * SCALE; y2 = rois[:, 4] * SCALE
    bw = np.maximum(x2 - x1, 1.0) / P
    bh = np.maximum(y2 - y1, 1.0) / P
    grid = np.arange(P, dtype=np.float32) + 0.5
    sx = x1[:, None, None] + bw[:, None, None] * grid[None, None, :]   # [R, P(py), P(px)]
    sy = y1[:, None, None] + bh[:, None, None] * grid[None, :, None]
    sx = np.broadcast_to(sx, (nroi, P, P)).reshape(-1)
    sy = np.broadcast_to(sy, (nroi, P, P)).reshape(-1)
    x0f = np.clip(np.floor(sx), 0, W - 1)
    y0f = np.clip(np.floor(sy), 0, H - 1)
    lx = np.clip(sx - x0f, 0.0, 1.0)
    ly = np.clip(sy - y0f, 0.0, 1.0)
    # clamp x0 to <= 30 adjusting lx (exact when sx>=31: both corners read col 31)
    x0 = x0f.astype(np.int32); y0 = y0f.astype(np.int32)
    hi = x0 >= 31
    x0 = np.where(hi, 30, x0); lx = np.where(hi, 1.0, lx).astype(np.float32)
    hiy = y0 >= 31
    y0 = np.where(hiy, 30, y0); ly = np.where(hiy, 1.0, ly).astype(np.float32)
    img = np.repeat(np.arange(nroi, dtype=np.int32) // N, 16)
    j2 = x0 >> 1
    par = (x0 & 1).astype(np.float32)
    idx0 = img * 512 + y0 * 16 + j2          # row idx (rows of 128 els)
    idx1 = idx0 + 16                          # y0+1 row
    w4 = np.stack([(1 - ly) * (1 - lx), (1 - ly) * lx, ly * (1 - lx), ly * lx], 1).astype(np.float32)
    return np.stack([idx0, idx1], 1).astype(np.int32), w4, par


def roi_w6_host(rois):
    """6 bilinear weights per point for the 2x(3 slot) gathered layout."""
    idx, w4, par = roi_grid_host(rois)
    wy0 = w4[:, 0] + w4[:, 1]          # (1-ly)
    wy1 = w4[:, 2] + w4[:, 3]          # ly
    lx = np.where(wy0 > 0, w4[:, 1] / np.maximum(wy0, 1e-30), w4[:, 3] / np.maximum(wy1, 1e-30))
    wa = (1 - par) * (1 - lx)
    wb = (1 - par) * lx + par * (1 - lx)
    wc = par * lx
    w6 = np.stack([wy0 * wa, wy0 * wb, wy0 * wc, wy1 * wa, wy1 * wb, wy1 * wc], 1)
    return idx, w6.astype(np.float32)


def feat2_rows_host(feat2r):
    rows = np.transpose(feat2r.reshape(-1, 64, 32, 16, 2), (0, 2, 3, 4, 1)).reshape(-1, 128)
    return np.concatenate([rows, np.zeros((2, 128), rows.dtype)], 0)


def roi_align_host(feat2r, rois):
    """Mirror of device gather+bilinear -> pooled [NPT, 64] pt-major."""
    idx, w6 = roi_w6_host(rois)
    rows = feat2_rows_host(feat2r)
    g = rows.reshape(-1)
    npt = idx.shape[0]
    blk = np.empty((npt, 2, 256), np.float32)
    for r in range(2):
        for p in range(npt):
            st = idx[p, r] * 128
            blk[p, r] = g[st: st + 256]
    offs = [0, 64, 128, 256, 320, 384]
    b2 = blk.reshape(npt, 512)
    pooled = np.zeros((npt, 64), np.float32)
    for s in range(6):
        pooled += w6[:, s:s + 1] * b2[:, offs[s]: offs[s] + 64]
    return pooled


# ---------------- GNN (host mirror of device algebra) ----------------
def mask_host(coor, r):
    """coor [BC, N, 2], r [BC, N] -> bigmask [NROW, NROW] fp32 block-diag, deg [NROW]."""
    bm = np.zeros((NROW, NROW), np.float32)
    for b in range(BC):
        d = np.linalg.norm(coor[b][:, None, :] - coor[b][None, :, :], axis=-1)
        m = (d <= (r[b][:, None] + r[b][None, :])) & ~np.eye(N, dtype=bool)
        bm[b * N:(b + 1) * N, b * N:(b + 1) * N] = m
    return bm, bm.sum(1)


def internet_host(s, bm, deg, p):
    """s [NROW, D] fp32 row-major; bm [NROW,NROW]; p = (sw,sb,rw,rb,aw,ab,ow,ob)."""
    sw, sb, rw, rb, aw, ab, ow, ob = p
    Wl, Wr = rw[:, :D], rw[:, D:]
    self_d = s @ sw.T + sb
    u = s @ Wl.T + rb
    v = s @ Wr.T
    rel = deg[:, None] * u + bm @ v
    a = np.maximum((self_d + rel) @ aw.T + ab, 0)
    return np.maximum(a @ ow[:, :D].T + s @ ow[:, D:].T + ob, 0)


def gnn_host(obj_t, src_coor, r, inputs):
    """obj_t [4][NROW, D] initial states; src_coor [BC, T, N, 2]; r [BC, N].
    Returns bboxes [BC, 8, N, 4]."""
    states = list(obj_t)
    masks = [mask_host(src_coor[:, t], r) for t in range(4)]
    num_rollouts = int(inputs['num_rollouts'])
    out = []
    for rr in range(num_rollouts):
        cs = []
        for k in range(4):
            p = (inputs['g_self_w'][k], inputs['g_self_b'][k], inputs['g_rel_w'][k],
                 inputs['g_rel_b'][k], inputs['g_aff_w'][k], inputs['g_aff_b'][k],
                 inputs['g_out_w'][k], inputs['g_out_b'][k])
            bm, deg = masks[k]
            cs.append(internet_host(states[k], bm, deg, p))
        s = np.concatenate(cs, -1) @ inputs['agg_w'].T + inputs['agg_b']
        bbox = s @ inputs['dec_w'].T + inputs['dec_b']          # [NROW, 4]
        out.append(bbox.reshape(BC, N, 4))
        states = states[1:] + [s]
        coor = bbox[:, 2:].reshape(BC, N, 2)
        masks = masks[1:] + [mask_host(coor, r)]
    return np.stack(out, 1)


def full_host(inputs, shard):
    """Complete per-core mirror (fp32). shard = B-slice index."""
    sl = slice(shard * BC, (shard + 1) * BC)
    x = inputs['x'][sl].reshape(NIMG, CIN, IMG, IMG)
    rois = inputs['rois'][sl].reshape(NROI, 5)
    coor = inputs['src_coor_features'][sl]                      # [BC, T, N, 2]
    r = (((rois.reshape(BC, T, N, 5)[..., 4] - rois.reshape(BC, T, N, 5)[..., 2]) / 2
          + (rois.reshape(BC, T, N, 5)[..., 3] - rois.reshape(BC, T, N, 5)[..., 1]) / 2) / 2).mean(1)
    f1 = np.maximum(conv1_host(x, inputs['w_conv1'], inputs['b_conv1']), 0)
    f2 = np.maximum(conv2_host(f1, inputs['w_conv2'], inputs['b_conv2']), 0)
    pooled = roi_align_host(f2, rois)                           # [NPT, 64] pt-major
    # fc0: obj[row, d] = sum_{c,pt} pool[row, pt, c] * fc0_w[d, c*16+pt]
    pool_cp = pooled.reshape(NROI, 16, 64)
    Wp = inputs['fc0_w'].reshape(D, 64, 16)                     # [d, c, pt]
    obj = np.einsum('rpc,dcp->rd', pool_cp, Wp) + inputs['fc0_b']
    obj = np.maximum(obj, 0)                                    # [NROI, D] rows (b,t,n)
    emb = np.maximum(coor.reshape(NROI, 2) @ inputs['fc0c_w'].T + inputs['fc0c_b'], 0)
    emb = np.maximum(emb @ inputs['fc1c_w'].T + inputs['fc1c_b'], 0)
    o2 = np.maximum(obj @ inputs['red_w'][:, :D].T + emb @ inputs['red_w'][:, D:].T
                    + inputs['red_b'], 0)                       # [NROI, D]
    o2 = o2.reshape(BC, T, N, D)
    obj_t = [o2[:, t].reshape(NROW, D) for t in range(4)]
    return gnn_host(obj_t, coor, r, inputs)


# ---------------- device input packing ----------------
def make_core_inputs(inputs, shard):
    import ml_dtypes
    bf16 = ml_dtypes.bfloat16
    sl = slice(shard * BC, (shard + 1) * BC)
    x = np.asarray(inputs['x'][sl], np.float32).reshape(NIMG, CIN, IMG, IMG)
    rois = np.asarray(inputs['rois'][sl], np.float32).reshape(NROI, 5)
    coor = np.asarray(inputs['src_coor_features'][sl], np.float32)   # [BC,T,N,2]
    rr5 = rois.reshape(BC, T, N, 5)
    r = (((rr5[..., 4] - rr5[..., 2]) / 2 + (rr5[..., 3] - rr5[..., 1]) / 2) / 2).mean(1)

    d = {}
    cols = conv1_im2col_host(x)                       # [45, NIMG, 64, 32]
    d['im2col45'] = cols.reshape(45, -1).astype(bf16)
    d['w1'] = conv1_weights_host(np.asarray(inputs['w_conv1'])).astype(bf16)
    b1 = np.asarray(inputs['b_conv1'], np.float32)
    d['b1'] = np.tile(b1, 2).reshape(128, 1).astype(np.float32)
    Wb = conv2_weights_host(np.asarray(inputs['w_conv2']))
    d['w2a'] = np.stack([Wb[dy][0] for dy in range(3)]).astype(bf16)
    d['w2b'] = np.stack([Wb[dy][1] for dy in range(3)]).astype(bf16)
    d['w2c'] = np.stack([Wb[dy][2] for dy in range(3)]).astype(bf16)
    b2 = np.asarray(inputs['b_conv2'], np.float32)
    d['b2'] = np.tile(b2, 2).reshape(128, 1).astype(np.float32)

    idx, w6 = roi_w6_host(rois)                       # [NPT,2] int32, [NPT,6]
    # device layout: [128 part, 24 groups] -- point p of group g = pt index g*128+p
    d['gidx'] = idx.reshape(NG, 128, 2).transpose(1, 0, 2).reshape(128, NG * 2).copy()
    d['w6'] = w6.reshape(NG, 128, 6).transpose(1, 0, 2).reshape(128, NG * 6).astype(np.float32)

    fc0w = np.asarray(inputs['fc0_w'], np.float32).reshape(D, 64, 16)  # [d, c, pt]
    d['fc0t'] = np.ascontiguousarray(fc0w.transpose(2, 1, 0)).astype(bf16)  # [pt, c, d]
    d['fc0b'] = np.asarray(inputs['fc0_b'], np.float32).reshape(2, 128).T.copy()

    d['coor_fm'] = coor.reshape(NROI, 2).T.astype(bf16).copy()

    def t2(w):   # [256, K] -> [kc, 128, 256] lhsT chunks (w.T row-chunks)
        wT = np.ascontiguousarray(np.asarray(w, np.float32).T)       # [K, 256]
        K = wT.shape[0]
        return wT.reshape(K // 128, 128, 256).astype(bf16)

    def bcol(b):  # [256] -> [128, 2]
        return np.asarray(b, np.float32).reshape(2, 128).T.copy()

    d['fc0ct'] = np.asarray(inputs['fc0c_w'], np.float32).T.astype(bf16).copy()  # [2, 256]
    d['fc0cb'] = bcol(inputs['fc0c_b'])
    d['fc1ct'] = t2(inputs['fc1c_w'])
    d['fc1cb'] = bcol(inputs['fc1c_b'])
    redw = np.asarray(inputs['red_w'], np.float32)
    d['redoT'] = t2(redw[:, :D])
    d['redeT'] = t2(redw[:, D:])
    d['redb'] = bcol(inputs['red_b'])

    d['gswT'] = np.stack([t2(inputs['g_self_w'][k]) for k in range(4)])
    grw = np.asarray(inputs['g_rel_w'], np.float32)
    d['gWlT'] = np.stack([t2(grw[k][:, :D]) for k in range(4)])
    d['gWrT'] = np.stack([t2(grw[k][:, D:]) for k in range(4)])
    d['gawT'] = np.stack([t2(inputs['g_aff_w'][k]) for k in range(4)])
    gow = np.asarray(inputs['g_out_w'], np.float32)
    d['gowaT'] = np.stack([t2(gow[k][:, :D]) for k in range(4)])
    d['gowsT'] = np.stack([t2(gow[k][:, D:]) for k in range(4)])
    d['gsb'] = np.stack([bcol(inputs['g_self_b'][k]) for k in range(4)])
    d['grb'] = np.stack([bcol(inputs['g_rel_b'][k]) for k in range(4)])
    d['gab'] = np.stack([bcol(inputs['g_aff_b'][k]) for k in range(4)])
    d['gob'] = np.stack([bcol(inputs['g_out_b'][k]) for k in range(4)])
    d['aggT'] = t2(inputs['agg_w'])                    # [8, 128, 256]
    d['aggb'] = bcol(inputs['agg_b'])
    decw = np.asarray(inputs['dec_w'], np.float32)     # [4, 256]
    d['decT'] = decw.T.reshape(2, 128, 4).astype(bf16).copy()
    d['decb'] = np.asarray(inputs['dec_b'], np.float32).reshape(4, 1).copy()

    hms, hds = [], []
    for m in range(4):
        bm, deg = mask_host(coor[:, m], r)
        hms.append(bm.astype(bf16))
        hds.append(np.broadcast_to(deg[None, :], (128, NROW)).astype(np.float32))
    d['hm'] = np.stack(hms)
    d['hdeg'] = np.ascontiguousarray(np.stack(hds))
    Tmat = np.full((NROW, NROW), -1.0, np.float32)
    for b in range(BC):
        rs = (r[b][:, None] + r[b][None, :]) ** 2
        np.fill_diagonal(rs, -1.0)
        Tmat[b * N:(b + 1) * N, b * N:(b + 1) * N] = rs
    d['Tm'] = Tmat
    d['ones48'] = np.ones((48, 128), bf16)
    d['ones2'] = np.ones((2, 48), bf16)
    d['decb2'] = np.asarray(inputs['dec_b'], np.float32)[2:4].reshape(2, 1).copy()
    d['ident'] = np.eye(128, dtype=bf16)
    return d


dt = mybir.dt
AF = mybir.ActivationFunctionType
OP = mybir.AluOpType

NIMG, NROI, NROW, NPT = 32, 192, 48, 3072
NG = 24            # gather groups (128 pts each)
IMG_GRP = 8        # images per conv group
NGRP = NIMG // IMG_GRP
IMGF = 2 * 33 * 33  # 2178 free els per img in feat1_ph


def build(nc: bass.Bass, dump=False):
    f32, bf16, i32 = dt.float32, dt.bfloat16, dt.int32

    def din(name, shape, d):
        return nc.dram_tensor(name, shape, d, kind="ExternalInput")

    im2col = din("im2col45", [45, 65536], bf16)
    w1 = din("w1", [45, 128], bf16)
    b1 = din("b1", [128, 1], f32)
    w2a = din("w2a", [3, 128, 128], bf16)
    w2b = din("w2b", [3, 128, 128], bf16)
    w2c = din("w2c", [3, 64, 128], bf16)
    b2 = din("b2", [128, 1], f32)
    gidx = din("gidx", [128, 48], i32)
    w6 = din("w6", [128, 144], f32)
    fc0t = din("fc0t", [16, 64, 256], bf16)
    fc0b = din("fc0b", [128, 2], f32)
    coor = din("coor_fm", [2, 192], bf16)
    fc0ct = din("fc0ct", [2, 256], bf16)
    fc0cb = din("fc0cb", [128, 2], f32)
    fc1ct = din("fc1ct", [2, 128, 256], bf16)
    fc1cb = din("fc1cb", [128, 2], f32)
    redoT = din("redoT", [2, 128, 256], bf16)
    redeT = din("redeT", [2, 128, 256], bf16)
    redb = din("redb", [128, 2], f32)
    gswT = din("gswT", [4, 2, 128, 256], bf16)
    gWlT = din("gWlT", [4, 2, 128, 256], bf16)
    gWrT = din("gWrT", [4, 2, 128, 256], bf16)
    gawT = din("gawT", [4, 2, 128, 256], bf16)
    gowaT = din("gowaT", [4, 2, 128, 256], bf16)
    gowsT = din("gowsT", [4, 2, 128, 256], bf16)
    gsb = din("gsb", [4, 128, 2], f32)
    grb = din("grb", [4, 128, 2], f32)
    gab = din("gab", [4, 128, 2], f32)
    gob = din("gob", [4, 128, 2], f32)
    aggT = din("aggT", [8, 128, 256], bf16)
    aggb = din("aggb", [128, 2], f32)
    decT = din("decT", [2, 128, 4], bf16)
    decb = din("decb", [4, 1], f32)
    hm = din("hm", [4, 48, 48], bf16)
    hdeg = din("hdeg", [4, 128, 48], f32)
    Tm = din("Tm", [48, 48], f32)
    ones48 = din("ones48", [48, 128], bf16)
    ones2 = din("ones2", [2, 48], bf16)
    decb2 = din("decb2", [2, 1], f32)
    ident = din("ident", [128, 128], bf16)

    out = nc.dram_tensor("bbox_out", [8, 8, 6, 4], f32, kind="ExternalOutput")
    if dump:
        d_f2 = nc.dram_tensor("d_f2", [16386, 128], bf16, kind="ExternalOutput")
        d_pool = nc.dram_tensor("d_pool", [64, 3072], bf16, kind="ExternalOutput")
        d_o2 = nc.dram_tensor("d_o2", [128, 384], bf16, kind="ExternalOutput")

    with tile.TileContext(nc) as tc, ExitStack() as ctx:
        # ---- persistent pools ----
        wp = ctx.enter_context(tc.tile_pool(name="w", bufs=1))
        dramp = ctx.enter_context(tc.tile_pool(name="dram", bufs=1, space="DRAM"))
        sp = ctx.enter_context(tc.tile_pool(name="state", bufs=1))

        def load(dram_t, shape, dtype, src_ap=None):
            t = wp.tile(shape, dtype, tag=dram_t.name)
            if src_ap is None:
                nc.sync.dma_start(t[:], dram_t[:, :])
            else:
                # src_ap dims [p, d0, d1, ...]; dst = t reshaped to match
                dims = [c for _, c in src_ap.ap[1:]]
                spec = " ".join(f"d{i}" for i in range(len(dims)))
                kw = {f"d{i}": dims[i] for i in range(len(dims) - 1)}
                dv = t[:].rearrange(f"p ({spec}) -> p {spec}", **kw)
                nc.sync.dma_start(dv, src_ap)
            return t

        w1_s = load(w1, [45, 128], bf16)
        b1_s = load(b1, [128, 1], f32)
        w2a_s = load(w2a, [128, 3 * 128], bf16, w2a[:].rearrange("d p m -> p d m"))
        w2b_s = load(w2b, [128, 3 * 128], bf16, w2b[:].rearrange("d p m -> p d m"))
        w2c_s = load(w2c, [64, 3 * 128], bf16, w2c[:].rearrange("d p m -> p d m"))
        b2_s = load(b2, [128, 1], f32)
        gidx_s = load(gidx, [128, 48], i32)
        w6_s = load(w6, [128, 144], f32)
        fc0t_s = load(fc0t, [64, 16 * 256], bf16, fc0t[:].rearrange("t p m -> p t m"))
        fc0b_s = load(fc0b, [128, 2], f32)
        coor_s = load(coor, [2, 192], bf16)
        fc0ct_s = load(fc0ct, [2, 256], bf16)
        fc0cb_s = load(fc0cb, [128, 2], f32)
        fc1ct_s = load(fc1ct, [128, 512], bf16, fc1ct[:].rearrange("k p m -> p k m"))
        fc1cb_s = load(fc1cb, [128, 2], f32)
        redoT_s = load(redoT, [128, 512], bf16, redoT[:].rearrange("k p m -> p k m"))
        redeT_s = load(redeT, [128, 512], bf16, redeT[:].rearrange("k p m -> p k m"))
        redb_s = load(redb, [128, 2], f32)

        def loadg(t):  # [4,2,128,256] -> [128, 4*512]
            return load(t, [128, 2048], bf16, t[:].rearrange("h k p m -> p h k m"))
        gswT_s, gWlT_s, gWrT_s = loadg(gswT), loadg(gWlT), loadg(gWrT)
        gawT_s, gowaT_s, gowsT_s = loadg(gawT), loadg(gowaT), loadg(gowsT)
        gsb_s = load(gsb, [128, 8], f32, gsb[:].rearrange("h p m -> p h m"))
        grb_s = load(grb, [128, 8], f32, grb[:].rearrange("h p m -> p h m"))
        gab_s = load(gab, [128, 8], f32, gab[:].rearrange("h p m -> p h m"))
        gob_s = load(gob, [128, 8], f32, gob[:].rearrange("h p m -> p h m"))
        aggT_s = load(aggT, [128, 2048], bf16, aggT[:].rearrange("k p m -> p k m"))
        aggb_s = load(aggb, [128, 2], f32)
        decT_s = load(decT, [128, 8], bf16, decT[:].rearrange("k p m -> p k m"))
        decb_s = load(decb, [4, 1], f32)
        Tm_s = load(Tm, [48, 48], f32)
        ones48_s = load(ones48, [48, 128], bf16)
        ones2_s = load(ones2, [2, 48], bf16)
        decb2_s = load(decb2, [2, 1], f32)
        ident_s = load(ident, [128, 128], bf16)

        # mask + deg slots (11 coor indices max; 0..3 from host)
        mask_t = [sp.tile([48, 48], bf16, name=f"mask{m}", tag=f"mask{m}") for m in range(11)]
        deg_t = [sp.tile([128, 48], f32, name=f"deg{m}", tag=f"deg{m}") for m in range(11)]
        for m in range(4):
            nc.sync.dma_start(mask_t[m][:], hm[m])
            nc.sync.dma_start(deg_t[m][:], hdeg[m])

        st = [sp.tile([128, 96], bf16, name=f"st{m}", tag=f"st{m}") for m in range(12)]
        bbox_sb = sp.tile([4, 384], f32, tag="bbox")
        poolT = sp.tile([64, 3072], bf16, tag="poolT")
        fd = dramp.tile([16386, 128], bf16, tag="feat2")

        # ================= conv stage =================
        with ExitStack() as cvx:
            imcp = cvx.enter_context(tc.tile_pool(name="imc", bufs=2))
            f1p = cvx.enter_context(tc.tile_pool(name="f1", bufs=2))
            c1ps = cvx.enter_context(tc.tile_pool(name="c1ps", bufs=2, space="PSUM"))
            c2ps = cvx.enter_context(tc.tile_pool(name="c2ps", bufs=2, space="PSUM"))
            tps = cvx.enter_context(tc.tile_pool(name="tps", bufs=2, space="PSUM"))
            f2p = cvx.enter_context(tc.tile_pool(name="f2", bufs=3))

            for g in range(NGRP):
                imc = imcp.tile([45, IMG_GRP * 2048], bf16, tag="imc")
                nc.sync.dma_start(imc[:], im2col[:, g * 16384:(g + 1) * 16384])
                f1 = f1p.tile([128, IMG_GRP * IMGF], bf16, tag="f1")
                # zero halo strips (Y=32 row, X=32 col)
                f1v = f1[:].rearrange("p (i y x) -> p i y x", i=IMG_GRP, y=2 * 33, x=33)
                nc.gpsimd.memset(f1v[:, :, :, 32:33], 0.0)
                f1h = f1[:].rearrange("p (i py y x) -> p i py y x", i=IMG_GRP, py=2, y=33, x=33)
                nc.gpsimd.memset(f1h[:, :, :, 32:33, :], 0.0)
                for i in range(IMG_GRP):
                    # conv1: 4 matmuls of [45,128]x[45,512] -> psum [128,1024] x2
                    pv = []
                    for h in range(2):
                        ps = c1ps.tile([128, 1024], f32, tag="c1")
                        for q in range(2):
                            nc.tensor.matmul(ps[:, q * 512:(q + 1) * 512], lhsT=w1_s[:],
                                             rhs=imc[:, i * 2048 + h * 1024 + q * 512:
                                                     i * 2048 + h * 1024 + (q + 1) * 512],
                                             start=True, stop=True)
                        pv.append(ps)
                    # evac relu+bias, phase-split: psum cols (oy 32, j 32) per half
                    for h in range(2):
                        psv = pv[h][:].rearrange("p (y j) -> p y j", y=32)
                        for py in range(2):
                            dst = f1h[:, i, py, 16 * h:16 * h + 16, 0:32]
                            eng = nc.vector if py else nc.scalar
                            if py:
                                nc.vector.tensor_scalar(
                                    out=dst, in0=psv[:, py::2, :], scalar1=b1_s[:, 0:1],
                                    scalar2=0.0, op0=OP.add, op1=OP.max)
                            else:
                                nc.scalar.activation(out=dst, in_=psv[:, py::2, :],
                                                     func=AF.Relu, bias=b1_s[:, 0:1])
                for i in range(IMG_GRP):
                    # conv2: 9 matmuls -> psum [128, 512] cols (oy2 32, j2 16)
                    ps = c2ps.tile([128, 512], f32, tag="c2")
                    first = True
                    f1v5 = f1[:].rearrange("p (i py y x) -> p i py y x",
                                           i=IMG_GRP, py=2, y=33, x=33)
                    for dy in range(3):
                        py, yo = dy % 2, dy // 2
                        for gsel in range(3):
                            sl = f1v5[:, i, py, yo:yo + 32, gsel:gsel + 1]
                            rhs_ap = bass.AP(sl.tensor, sl.offset,
                                             [sl.ap[0], sl.ap[1], [2, 16]])
                            if gsel == 2:
                                rhs_ap = rhs_ap[0:64]
                                lhsT = w2c_s[:, dy * 128:(dy + 1) * 128]
                            else:
                                lhsT = (w2a_s if gsel == 0 else w2b_s)[:, dy * 128:(dy + 1) * 128]
                            nc.tensor.matmul(ps[:], lhsT=lhsT, rhs=rhs_ap,
                                             start=first, stop=(dy == 2 and gsel == 2))
                            first = False
                    f2s = f2p.tile([128, 512], bf16, tag="f2s")
                    if i % 2 == 0:
                        nc.vector.tensor_scalar(out=f2s[:], in0=ps[:], scalar1=b2_s[:, 0:1],
                                                scalar2=0.0, op0=OP.add, op1=OP.max)
                    else:
                        nc.scalar.activation(out=f2s[:], in_=ps[:], func=AF.Relu,
                                             bias=b2_s[:, 0:1])
                    tp = tps.tile([128, 512], bf16, tag="tp")
                    for b in range(4):
                        nc.tensor.transpose(tp[:, b * 128:(b + 1) * 128],
                                            f2s[:, b * 128:(b + 1) * 128], ident_s[:])
                    f2t = f2p.tile([128, 512], bf16, tag="f2t")
                    if i % 2 == 0:
                        nc.scalar.activation(out=f2t[:], in_=tp[:], func=AF.Copy)
                    else:
                        nc.vector.tensor_copy(out=f2t[:], in_=tp[:])
                    img = g * IMG_GRP + i
                    dst = fd[img * 512:(img + 1) * 512, :].rearrange(
                        "(b p) c -> p b c", p=128)
                    nc.sync.dma_start(dst, f2t[:].rearrange("p (b c) -> p b c", c=128))
                    if dump:
                        dd = d_f2[img * 512:(img + 1) * 512, :].rearrange(
                            "(b p) c -> p b c", p=128)
                        nc.sync.dma_start(dd, f2t[:].rearrange("p (b c) -> p b c", c=128))

        # ================= roi gather + fc0 + emb =================
        with ExitStack() as gx:
            gp = gx.enter_context(tc.tile_pool(name="g", bufs=3))
            bp = gx.enter_context(tc.tile_pool(name="bil", bufs=3))
            ptps = gx.enter_context(tc.tile_pool(name="ptps", bufs=2, space="PSUM"))
            ops = gx.enter_context(tc.tile_pool(name="ops", bufs=2, space="PSUM"))

            fdv = fd[:]  # [16386, 128]
            for g in range(NG):
                gb = gp.tile([128, 512], bf16, tag="gb")
                for rrow in range(2):
                    nc.gpsimd.indirect_dma_start(
                        out=gb[:, rrow * 256:(rrow + 1) * 256], out_offset=None, in_=fdv,
                        in_offset=bass.IndirectOffsetOnAxis(
                            ap=gidx_s[:, 2 * g + rrow:2 * g + rrow + 1], axis=0))
                offs = [0, 64, 128, 256, 320, 384]
                a0 = bp.tile([128, 64], f32, tag="acc0")
                a1 = bp.tile([128, 64], f32, tag="acc1")
                nc.vector.tensor_scalar(out=a0[:], in0=gb[:, 0:64],
                                        scalar1=w6_s[:, 6 * g:6 * g + 1], scalar2=None, op0=OP.mult)
                cur, nxt = a0, a1
                for s in range(1, 6):
                    dst = bp.tile([128, 64], bf16, name="pb", tag="pb") if s == 5 else nxt
                    nc.vector.scalar_tensor_tensor(
                        out=dst[:], in0=gb[:, offs[s]:offs[s] + 64],
                        scalar=w6_s[:, 6 * g + s:6 * g + s + 1], in1=cur[:],
                        op0=OP.mult, op1=OP.add)
                    if s < 5:
                        cur, nxt = dst, cur
                pb = dst
                pt = ptps.tile([64, 128], bf16, tag="pt")
                nc.tensor.transpose(pt[:], pb[:], ident_s[:])
                nc.scalar.activation(out=poolT[:, 128 * g:128 * (g + 1)], in_=pt[:], func=AF.Copy)
            if dump:
                nc.sync.dma_start(d_pool[:, :], poolT[:])

            obj = sp.tile([128, 384], bf16, tag="obj")
            pview = poolT[:].rearrange("p (r t) -> p t r", t=16)
            for m2 in range(2):
                ps = ops.tile([128, 192], f32, tag="obj")
                for pt_i in range(16):
                    nc.tensor.matmul(ps[:], lhsT=fc0t_s[:, pt_i * 256 + m2 * 128:
                                                        pt_i * 256 + m2 * 128 + 128],
                                     rhs=pview[:, pt_i, :],
                                     start=(pt_i == 0), stop=(pt_i == 15))
                nc.scalar.activation(out=obj[:, m2 * 192:(m2 + 1) * 192], in_=ps[:],
                                     func=AF.Relu, bias=fc0b_s[:, m2:m2 + 1])
            emb1 = sp.tile([128, 384], bf16, tag="emb1")
            for m2 in range(2):
                ps = ops.tile([128, 192], f32, tag="emb")
                nc.tensor.matmul(ps[:], lhsT=fc0ct_s[:, m2 * 128:(m2 + 1) * 128],
                                 rhs=coor_s[:], start=True, stop=True)
                nc.scalar.activation(out=emb1[:, m2 * 192:(m2 + 1) * 192], in_=ps[:],
                                     func=AF.Relu, bias=fc0cb_s[:, m2:m2 + 1])
            emb2 = sp.tile([128, 384], bf16, tag="emb2")
            for m2 in range(2):
                ps = ops.tile([128, 192], f32, tag="emb")
                for kc in range(2):
                    nc.tensor.matmul(ps[:], lhsT=fc1ct_s[:, kc * 256 + m2 * 128:
                                                         kc * 256 + m2 * 128 + 128],
                                     rhs=emb1[:, kc * 192:(kc + 1) * 192],
                                     start=(kc == 0), stop=(kc == 1))
                nc.scalar.activation(out=emb2[:, m2 * 192:(m2 + 1) * 192], in_=ps[:],
                                     func=AF.Relu, bias=fc1cb_s[:, m2:m2 + 1])
            o2 = sp.tile([128, 384], bf16, tag="o2")
            for m2 in range(2):
                ps = ops.tile([128, 192], f32, tag="o2")
                for kc in range(2):
                    nc.tensor.matmul(ps[:], lhsT=redoT_s[:, kc * 256 + m2 * 128:
                                                         kc * 256 + m2 * 128 + 128],
                                     rhs=obj[:, kc * 192:(kc + 1) * 192],
                                     start=(kc == 0), stop=False)
                for kc in range(2):
                    nc.tensor.matmul(ps[:], lhsT=redeT_s[:, kc * 256 + m2 * 128:
                                                         kc * 256 + m2 * 128 + 128],
                                     rhs=emb2[:, kc * 192:(kc + 1) * 192],
                                     start=False, stop=(kc == 1))
                nc.scalar.activation(out=o2[:, m2 * 192:(m2 + 1) * 192], in_=ps[:],
                                     func=AF.Relu, bias=redb_s[:, m2:m2 + 1])
            if dump:
                nc.sync.dma_start(d_o2[:, :], o2[:])
            # initial states: s_m [128, 96] cols m2*48 + b*6 + n  <- o2 cols m2*192 + b*24 + m*6 + n
            o2v = o2[:].rearrange("p (m2 b t n) -> p m2 b t n", m2=2, b=8, t=4)
            for m in range(4):
                nc.vector.tensor_copy(
                    out=st[m][:].rearrange("p (m2 b n) -> p m2 b n", m2=2, b=8),
                    in_=o2v[:, :, :, m, :])

        # ================= GNN rollouts =================
        with ExitStack() as rx:
            gps = rx.enter_context(tc.tile_pool(name="gps", bufs=4, space="PSUM"))
            vps = rx.enter_context(tc.tile_pool(name="vps", bufs=1, space="PSUM"))
            sps = rx.enter_context(tc.tile_pool(name="sps", bufs=2, space="PSUM"))
            hb = rx.enter_context(tc.tile_pool(name="hbuf", bufs=3))

            for rr in range(8):
                cs = []
                for k in range(4):
                    m = rr + k
                    s = st[m]
                    W = slice(k * 512, (k + 1) * 512)
                    u_ps = gps.tile([128, 96], f32, tag="g")
                    sd_ps = gps.tile([128, 96], f32, tag="g")
                    rel_ps = gps.tile([128, 96], f32, tag="g")
                    for m2 in range(2):
                        for kc in range(2):
                            lo = k * 512 + kc * 256 + m2 * 128
                            nc.tensor.matmul(u_ps[:, m2 * 48:m2 * 48 + 48],
                                             lhsT=gWlT_s[:, lo:lo + 128],
                                             rhs=s[:, kc * 48:kc * 48 + 48],
                                             start=(kc == 0), stop=(kc == 1))
                            nc.tensor.matmul(sd_ps[:, m2 * 48:m2 * 48 + 48],
                                             lhsT=gswT_s[:, lo:lo + 128],
                                             rhs=s[:, kc * 48:kc * 48 + 48],
                                             start=(kc == 0), stop=(kc == 1))
                    v_ps = vps.tile([48, 256], f32, tag="v")
                    for kc in range(2):
                        nc.tensor.matmul(v_ps[:], lhsT=s[:, kc * 48:kc * 48 + 48],
                                         rhs=gWrT_s[:, k * 512 + kc * 256:
                                                    k * 512 + (kc + 1) * 256],
                                         start=(kc == 0), stop=(kc == 1))
                    v_sb = hb.tile([48, 256], bf16, tag="v")
                    nc.vector.tensor_copy(out=v_sb[:], in_=v_ps[:])
                    for m2 in range(2):
                        nc.tensor.matmul(rel_ps[:, m2 * 48:m2 * 48 + 48],
                                         lhsT=v_sb[:, m2 * 128:(m2 + 1) * 128],
                                         rhs=mask_t[m][:], start=True, stop=True)
                    x_sb = hb.tile([128, 96], bf16, tag="x")
                    for m2 in range(2):
                        h = slice(m2 * 48, m2 * 48 + 48)
                        t1 = hb.tile([128, 48], f32, tag="t1")
                        nc.vector.tensor_scalar(out=t1[:], in0=u_ps[:, h],
                                                scalar1=grb_s[:, 2 * k + m2:2 * k + m2 + 1],
                                                scalar2=None, op0=OP.add)
                        t2 = hb.tile([128, 48], f32, tag="t2")
                        nc.vector.tensor_tensor(out=t2[:], in0=t1[:], in1=deg_t[m][:], op=OP.mult)
                        t3 = hb.tile([128, 48], f32, tag="t3")
                        nc.vector.scalar_tensor_tensor(
                            out=t3[:], in0=sd_ps[:, h],
                            scalar=gsb_s[:, 2 * k + m2:2 * k + m2 + 1], in1=t2[:],
                            op0=OP.add, op1=OP.add)
                        nc.vector.tensor_tensor(out=x_sb[:, h], in0=t3[:], in1=rel_ps[:, h],
                                                op=OP.add)
                    a_ps = gps.tile([128, 96], f32, tag="g")
                    for m2 in range(2):
                        for kc in range(2):
                            lo = k * 512 + kc * 256 + m2 * 128
                            nc.tensor.matmul(a_ps[:, m2 * 48:m2 * 48 + 48],
                                             lhsT=gawT_s[:, lo:lo + 128],
                                             rhs=x_sb[:, kc * 48:kc * 48 + 48],
                                             start=(kc == 0), stop=(kc == 1))
                    a_sb = hb.tile([128, 96], bf16, tag="a")
                    for m2 in range(2):
                        nc.scalar.activation(out=a_sb[:, m2 * 48:m2 * 48 + 48],
                                             in_=a_ps[:, m2 * 48:m2 * 48 + 48], func=AF.Relu,
                                             bias=gab_s[:, 2 * k + m2:2 * k + m2 + 1])
                    o_ps = gps.tile([128, 96], f32, tag="g")
                    for m2 in range(2):
                        for kc in range(2):
                            lo = k * 512 + kc * 256 + m2 * 128
                            nc.tensor.matmul(o_ps[:, m2 * 48:m2 * 48 + 48],
                                             lhsT=gowaT_s[:, lo:lo + 128],
                                             rhs=a_sb[:, kc * 48:kc * 48 + 48],
                                             start=(kc == 0), stop=False)
                            nc.tensor.matmul(o_ps[:, m2 * 48:m2 * 48 + 48],
                                             lhsT=gowsT_s[:, lo:lo + 128],
                                             rhs=s[:, kc * 48:kc * 48 + 48],
                                             start=False, stop=(kc == 1))
                    c_sb = hb.tile([128, 96], bf16, tag=f"cs{k}")
                    for m2 in range(2):
                        nc.scalar.activation(out=c_sb[:, m2 * 48:m2 * 48 + 48],
                                             in_=o_ps[:, m2 * 48:m2 * 48 + 48], func=AF.Relu,
                                             bias=gob_s[:, 2 * k + m2:2 * k + m2 + 1])
                    cs.append(c_sb)
                g_ps = gps.tile([128, 96], f32, tag="g")
                for m2 in range(2):
                    n = 0
                    for k in range(4):
                        for kc in range(2):
                            lo = (k * 2 + kc) * 256 + m2 * 128
                            nc.tensor.matmul(g_ps[:, m2 * 48:m2 * 48 + 48],
                                             lhsT=aggT_s[:, lo:lo + 128],
                                             rhs=cs[k][:, kc * 48:kc * 48 + 48],
                                             start=(n == 0), stop=(n == 7))
                            n += 1
                s_new = st[rr + 4]
                for m2 in range(2):
                    nc.vector.tensor_scalar(out=s_new[:, m2 * 48:m2 * 48 + 48],
                                            in0=g_ps[:, m2 * 48:m2 * 48 + 48],
                                            scalar1=aggb_s[:, m2:m2 + 1], scalar2=None,
                                            op0=OP.add)
                d_ps = sps.tile([4, 48], f32, tag="s")
                for kc in range(2):
                    nc.tensor.matmul(d_ps[:], lhsT=decT_s[:, kc * 4:kc * 4 + 4],
                                     rhs=s_new[:, kc * 48:kc * 48 + 48],
                                     start=(kc == 0), stop=(kc == 1))
                bbv = bbox_sb[:].rearrange("f (b q) -> f b q", b=8)[:, :, rr * 6:rr * 6 + 6]
                nc.vector.tensor_scalar(out=bbv, in0=d_ps[:],
                                        scalar1=decb_s[:, 0:1], scalar2=None, op0=OP.add)
                if rr < 7:
                    m = rr + 4
                    d2_ps = sps.tile([2, 48], f32, tag="s")
                    for kc in range(2):
                        nc.tensor.matmul(d2_ps[:], lhsT=decT_s[:, kc * 4 + 2:kc * 4 + 4],
                                         rhs=s_new[:, kc * 48:kc * 48 + 48],
                                         start=(kc == 0), stop=(kc == 1))
                    coorb = hb.tile([2, 48], bf16, tag="coorb")
                    nc.vector.tensor_scalar(out=coorb[:], in0=d2_ps[:],
                                            scalar1=decb2_s[:, 0:1], scalar2=None, op0=OP.add)
                    cm2 = hb.tile([2, 48], bf16, tag="cm2")
                    nc.vector.tensor_scalar(out=cm2[:], in0=coorb[:], scalar1=-2.0,
                                            scalar2=None, op0=OP.mult)
                    sq = hb.tile([2, 48], bf16, tag="sq")
                    nc.vector.tensor_tensor(out=sq[:], in0=coorb[:], in1=coorb[:], op=OP.mult)
                    m_ps = sps.tile([48, 48], f32, tag="s")
                    nc.tensor.matmul(m_ps[:], lhsT=coorb[:], rhs=cm2[:], start=True, stop=False)
                    nc.tensor.matmul(m_ps[:], lhsT=sq[:], rhs=ones2_s[:], start=False, stop=False)
                    nc.tensor.matmul(m_ps[:], lhsT=ones2_s[:], rhs=sq[:], start=False, stop=True)
                    nc.vector.tensor_tensor(out=mask_t[m][:], in0=m_ps[:], in1=Tm_s[:],
                                            op=OP.is_le)
                    dd_ps = sps.tile([128, 48], f32, tag="s")
                    nc.tensor.matmul(dd_ps[:], lhsT=ones48_s[:], rhs=mask_t[m][:],
                                     start=True, stop=True)
                    nc.vector.tensor_copy(out=deg_t[m][:], in_=dd_ps[:])
            nc.sync.dma_start(
                out[:].rearrange("b rr n f -> f (b rr n)"), bbox_sb[:])
    return nc


_NC = None

def _get_nc():
    global _NC
    if _NC is None:
        nc = bass.Bass()
        build(nc)
        split_drain_waits(nc)
        _NC = nc
    return _NC


def kernel(**inputs):
    nc = _get_nc()
    inputs = {k: np.asarray(v) for k, v in inputs.items()}
    maps = [make_core_inputs(inputs, s) for s in range(NCORE)]
    res = run_bass_kernel_spmd(nc, maps, core_ids=list(range(NCORE)))
    out = np.concatenate([res.results[s]["bbox_out"] for s in range(NCORE)], 0)
    return out.astype(np.float32)

